# revision 39
# baseline (speedup 1.0000x reference)
"""Trainium2 Bass kernel for nn_MultiHeadAttention_91027536871977.

Cosine-similarity multi-head self-attention:
  x      = einsum("bsd,hdf->bhsf", sin, Wx) + bx          [B,H,S,F]
  scores = (x @ x^T) / (|x| |x|^T)                        [B,H,S,S]
  p      = softmax(scores, -1)
  out    = concat_heads(p @ x) @ Wp + bp                  [B,S,D]

Sharding: pure data-parallel over batch (B=8 -> 8 cores, one batch each,
all 16 heads + the output projection local to the core; no collectives).

v3 (vs the 299us v1): v1's trace showed PE 80% busy dominated by ~280
small N=128 layout matmuls (transposes + selector broadcasts) and their
dependency stalls.  v3:
  - X [t,hf] via matmul as before (full-rate K=128/N=512 streams)
  - X^T per pair via a SECOND projection (lhsT = Wx blocks) instead of
    64 PE transposes; bias + 1/|x| column scaling fused into one DVE
    scalar_tensor_tensor on the PSUM result
  - the 1/|x| and 1/rs broadcast tiles ([f2,t] replication of per-token
    scalars) built by DRAM-roundtrip DMA broadcasts instead of 128
    selector matmuls on PE
  - gram/exp/out^T/Y structure kept from v1 (row-tiled K=64 gram pairs,
    col-tiled concurrent out^T pairs, exp on ACT with accum_out row sums)
  - bf16 output DMA (halves the output-drain tail; harness tolerance is
    2e-2, measured rel err ~4.4e-3)

Explored and measured SLOWER (see session notes): DVE exp polynomial in
any form (cast+accum ops get no DVE fast modes -> ~4.5-6us/tile vs ACT
1.46), quadrant 2x2 gram packing (group cadence unchanged), PSUM pool
splitting (serializes projections), xt prefetch reordering, ACT-side
squares.  HW power throttling (util capped to 0.5 for ~30-60% of
runtime, varying run to run) dominates fine-grained scheduling effects.
"""

import numpy as np
import ml_dtypes

import concourse.bass as bass
import concourse.bacc as bacc
import concourse.mybir as mybir
import concourse.tile as tile
from concourse.bass_utils import run_bass_kernel_spmd

B, S, D, H, F = 8, 1024, 1024, 16, 64
P = 128
NP = H // 2  # head pairs
KO = D // P  # k subtiles
NT = S // P  # s tiles
BF16 = mybir.dt.bfloat16
F32 = mybir.dt.float32
HALF = S // 2


def build_program() -> bass.Bass:
    nc = bacc.Bacc("TRN2", target_bir_lowering=False, debug=False)

    d_sint = nc.dram_tensor("sint", [D, S], BF16, kind="ExternalInput")
    d_wx = nc.dram_tensor("wx", [D, H * F], BF16, kind="ExternalInput")
    d_wp = nc.dram_tensor("wp", [H * F, D], BF16, kind="ExternalInput")
    d_bxp = nc.dram_tensor("bxp", [P, NP], F32, kind="ExternalInput")
    d_bxf = nc.dram_tensor("bxf", [1, H * F], F32, kind="ExternalInput")
    d_bp = nc.dram_tensor("bp", [1, D], F32, kind="ExternalInput")
    d_ident = nc.dram_tensor("ident", [P, P], BF16, kind="ExternalInput")
    d_bsc = nc.dram_tensor("bsc", [2, NP, 2 * NT, P], BF16, kind="Internal")
    d_y = nc.dram_tensor("y", [S, D], BF16, kind="ExternalOutput")

    with tile.TileContext(nc) as tc:
        _body(tc, d_sint, d_wx, d_wp, d_bxp, d_bxf, d_bp, d_ident,
              d_bsc, d_y)
    nc.compile()
    return nc


def _bcast_rows(dram_ap, parts=P, width=None):
    """DMA access pattern replicating a contiguous DRAM region across
    `parts` partitions (each partition reads the same `width` elements)."""
    if width is None:
        width = dram_ap.ap[-1][1]
    return bass.AP(
        tensor=dram_ap.tensor,
        offset=dram_ap.offset,
        ap=[[0, parts], [1, width]],
    )


def _body(tc, d_sint, d_wx, d_wp, d_bxp, d_bxf, d_bp, d_ident, d_bsc, d_y):
    nc = tc.nc
    from contextlib import ExitStack

    with ExitStack() as ctx:
        singles = ctx.enter_context(tc.tile_pool(name="singles", bufs=1))
        sq_pool = ctx.enter_context(tc.tile_pool(name="sq", bufs=2))
        e_pool = ctx.enter_context(tc.tile_pool(name="epool", bufs=4))
        b_pool = ctx.enter_context(tc.tile_pool(name="bpool", bufs=2))
        y_pool = ctx.enter_context(tc.tile_pool(name="ypool", bufs=2))
        bc_pool = ctx.enter_context(tc.tile_pool(name="bcpool", bufs=1))

        # PSUM (8 banks): ps_g 2x[P,S]f32 = 4 (gram ring + X-proj tiles,
        # which only exist in the prologue), ps_xt 1x2 = 2 (xt/Y tiles, so
        # they never steal gram-ring buffers -> no pair-boundary ACT
        # stalls), ps_small 2x1 = 2 (out^T halves, small transposes).
        ps_g = ctx.enter_context(tc.tile_pool(name="ps_g", bufs=2, space="PSUM"))
        ps_xt = ctx.enter_context(tc.tile_pool(name="ps_xt", bufs=1, space="PSUM"))
        ps_small = ctx.enter_context(tc.tile_pool(name="ps_small", bufs=2, space="PSUM"))

        # ---- load everything to SBUF ----
        sint_sb = singles.tile([P, KO, S], BF16)
        wx_sb = singles.tile([P, KO, H * F], BF16)
        sint_r = d_sint.rearrange("(ko p) s -> p ko s", p=P)
        wx_r = d_wx.rearrange("(ko p) n -> p ko n", p=P)
        for ko in range(KO):
            nc.sync.dma_start(wx_sb[:, ko, :], wx_r[:, ko, :])
            nc.sync.dma_start(sint_sb[:, ko, :], sint_r[:, ko, :])
        wp_sb = singles.tile([P, KO, D], BF16)
        nc.sync.dma_start(wp_sb, d_wp.rearrange("(ko p) n -> p ko n", p=P))
        bxf_sb = bc_pool.tile([P, H * F], F32, tag="bc", name="bxf_sb")
        nc.gpsimd.dma_start(bxf_sb, _bcast_rows(d_bxf[:, :]))
        bxp_sb = singles.tile([P, NP], F32)
        nc.sync.dma_start(bxp_sb, d_bxp[:, :])
        ident_sb = singles.tile([P, P], BF16)
        nc.sync.dma_start(ident_sb, d_ident[:, :])

        # persistent intermediates
        x_sb = singles.tile([P, NT, H * F], BF16)   # x [t_p, t_tile, hf]
        xtn_sb = singles.tile([P, NP, S], BF16)     # normalized x^T [f2, pair, t]
        nrm_sb = singles.tile([P, NP, S], BF16)     # 1/|x| bcast [f2, pair, t]
        outt_sb = singles.tile([P, NP, S], BF16)    # attention out^T
        rs_sb = singles.tile([P, P], F32)           # rs[s_p, col h*8+i]
        n2s_sb = singles.tile([P, P], F32)          # |x|^2 [s_p, col h*8+i]
        nrcp_sb = singles.tile([P, P], F32)         # 1/|x|^2 scratch
        invs_sb = singles.tile([P, P], BF16)        # 1/|x| [s_p, col h*8+i]

        # ---- Phase A: X = sin @ Wx + bx in [t, hf] layout, per-head |x|^2 ----
        for i in range(NT):
            x_ps = ps_g.tile([P, H * F], F32, tag="g", name=f"x_{i}")
            for hlf in range(2):
                for ko in range(KO):
                    nc.tensor.matmul(
                        x_ps[:, hlf * HALF:(hlf + 1) * HALF],
                        lhsT=sint_sb[:, ko, i * P:(i + 1) * P],
                        rhs=wx_sb[:, ko, hlf * HALF:(hlf + 1) * HALF],
                        start=(ko == 0), stop=(ko == KO - 1),
                    )
            nc.vector.tensor_add(x_sb[:, i, :], x_ps, bxf_sb[:, :])
            xsq = sq_pool.tile([P, H * F], BF16, tag="xsq", name=f"xsq_{i}")
            nc.vector.tensor_mul(xsq, x_sb[:, i, :], x_sb[:, i, :])
            nc.vector.reduce_sum(
                n2s_sb.rearrange("p (hh ii) -> p hh ii", ii=NT)[:, :, i],
                xsq.rearrange("p (hh f) -> p hh f", f=F),
                axis=mybir.AxisListType.X,
            )
        nc.vector.reciprocal(nrcp_sb, n2s_sb)
        nc.scalar.sqrt(invs_sb, nrcp_sb)

        # ---- Phase A2: per-pair 1/|x| broadcast tiles via DRAM roundtrip ----
        for q in range(NP):
            invq_ps = ps_small.tile([2 * NT, P], BF16, tag="small",
                                    name=f"invq_{q}")
            nc.tensor.transpose(
                invq_ps, invs_sb[:, q * 2 * NT:(q + 1) * 2 * NT], ident_sb)
            invq_sb = b_pool.tile([2 * NT, P], BF16, tag="rcpq",
                                  name=f"invqs_{q}")
            nc.vector.tensor_copy(invq_sb, invq_ps)
            nc.sync.dma_start(d_bsc[0, q, :, :], invq_sb)
            for hh in range(2):
                nc.gpsimd.dma_start(
                    nrm_sb[hh * F:(hh + 1) * F, q, :],
                    _bcast_rows(d_bsc[0, q, hh * NT:(hh + 1) * NT, :],
                                parts=F, width=S))

        e_store = {}

        def xt_build(q):
            """X^T for pair q via direct projection; bias + normalize fused."""
            xt_ps = ps_xt.tile([P, S], F32, tag="xt", name=f"xt_{q}")
            for hlf in range(2):
                for ko in range(KO):
                    nc.tensor.matmul(
                        xt_ps[:, hlf * HALF:(hlf + 1) * HALF],
                        lhsT=wx_sb[:, ko, q * P:(q + 1) * P],
                        rhs=sint_sb[:, ko, hlf * HALF:(hlf + 1) * HALF],
                        start=(ko == 0), stop=(ko == KO - 1),
                    )
            nc.vector.scalar_tensor_tensor(
                xtn_sb[:, q, :], xt_ps, bxp_sb[:, q:q + 1], nrm_sb[:, q, :],
                op0=mybir.AluOpType.add, op1=mybir.AluOpType.mult,
            )

        def gram_tile(q, i):
            """Gram + exp for both heads of pair q at s-tile i."""
            g_tiles = [
                ps_g.tile([P, S], F32, tag="g", name=f"g_{q}_{hh}_{i}")
                for hh in range(2)]
            for hlf in range(2):
                for hh in range(2):
                    frows = slice(hh * F, (hh + 1) * F)
                    nc.tensor.matmul(
                        g_tiles[hh][:, hlf * HALF:(hlf + 1) * HALF],
                        lhsT=xtn_sb[frows, q, i * P:(i + 1) * P],
                        rhs=xtn_sb[frows, q, hlf * HALF:(hlf + 1) * HALF],
                        start=True, stop=True,
                    )
            for hh in range(2):
                h = 2 * q + hh
                nc.scalar.activation(
                    e_store[q][hh][:, i, :], g_tiles[hh],
                    mybir.ActivationFunctionType.Exp,
                    accum_out=rs_sb[:, h * NT + i:h * NT + i + 1],
                )

        def gram(q):
            e_store[q] = [
                e_pool.tile([P, NT, S], BF16, tag="e", name=f"e_{q}_{hh}")
                for hh in range(2)]
            for i in range(NT):
                gram_tile(q, i)

        def rs_chain(q):
            """1/rs broadcast tile for pair q via DRAM roundtrip."""
            rcps_sb = b_pool.tile([P, 2 * NT], F32, tag="rcps",
                                  name=f"rcps_{q}")
            nc.vector.reciprocal(
                rcps_sb, rs_sb[:, q * 2 * NT:(q + 1) * 2 * NT])
            rcpsb_sb = b_pool.tile([P, 2 * NT], BF16, tag="rcpsb",
                                   name=f"rcpsb_{q}")
            nc.vector.tensor_copy(rcpsb_sb, rcps_sb)
            rst_ps = ps_small.tile([2 * NT, P], BF16, tag="small",
                                   name=f"rst_{q}")
            nc.tensor.transpose(rst_ps, rcpsb_sb, ident_sb)
            rcpq_sb = b_pool.tile([2 * NT, P], BF16, tag="rcpq",
                                  name=f"rcpq_{q}")
            nc.vector.tensor_copy(rcpq_sb, rst_ps)
            nc.sync.dma_start(d_bsc[1, q, :, :], rcpq_sb)
            brc_sb = b_pool.tile([P, S], BF16, tag="brc", name=f"brc_{q}")
            for hh in range(2):
                nc.gpsimd.dma_start(
                    brc_sb[hh * F:(hh + 1) * F, :],
                    _bcast_rows(d_bsc[1, q, hh * NT:(hh + 1) * NT, :],
                                parts=F, width=S))
            return brc_sb

        def ex_half(q, hlf, brc_sb, nxt, filler=None):
            """Half of out^T accumulation for pair q, with pair nxt's gram
            tiles (or a filler task) interleaved to keep ACT/PE fed."""
            ot_ps = ps_small.tile([P, HALF], F32, tag="small",
                                  name=f"ot_{q}_{hlf}")
            for j in range(NT):
                if nxt is not None and j % 2 == 0:
                    gram_tile(nxt, hlf * 4 + j // 2)
                elif filler is not None and j % 2 == 0:
                    filler(hlf * 4 + j // 2)
                for hh2 in range(2):
                    nc.tensor.matmul(
                        ot_ps[hh2 * F:(hh2 + 1) * F, :],
                        lhsT=x_sb[:, j, (2 * q + hh2) * F:(2 * q + hh2 + 1) * F],
                        rhs=e_store[q][hh2][:, j, hlf * HALF:(hlf + 1) * HALF],
                        start=(j == 0), stop=(j == NT - 1),
                        tile_position=(0, hh2 * F),
                        skip_group_check=True,
                    )
            nc.vector.tensor_mul(
                outt_sb[:, q, hlf * HALF:(hlf + 1) * HALF],
                brc_sb[:, hlf * HALF:(hlf + 1) * HALF],
                ot_ps,
            )

        # ---- software-pipelined attention over pairs ----
        bp_sb = bc_pool.tile([P, D], F32, tag="bc", name="bp_sb")
        nc.gpsimd.dma_start(bp_sb, _bcast_rows(d_bp[:, :]))
        y_part_ref = []

        def y_partial(i):
            """Output projection over pairs 0..6 for s-tile i, staged bf16;
            interleaved into pair 7's out^T phase where PE otherwise idles
            while ACT runs pair 7's exps."""
            yp_ps = ps_xt.tile([P, D], F32, tag="xt", name=f"yp_{i}")
            for hlf in range(2):
                for qq in range(NP - 1):
                    nc.tensor.matmul(
                        yp_ps[:, hlf * HALF:(hlf + 1) * HALF],
                        lhsT=outt_sb[:, qq, i * P:(i + 1) * P],
                        rhs=wp_sb[:, qq, hlf * HALF:(hlf + 1) * HALF],
                        start=(qq == 0), stop=(qq == NP - 2),
                    )
            nc.vector.tensor_add(y_part_ref[0][:, i, :], yp_ps, bp_sb)

        xt_build(0)
        gram(0)
        for q in range(NP):
            nxt = q + 1 if q + 1 < NP else None
            if nxt is not None:
                xt_build(nxt)
                e_store[nxt] = [
                    e_pool.tile([P, NT, S], BF16, tag="e", name=f"e_{nxt}_{hh}")
                    for hh in range(2)]
            else:
                y_part_ref.append(
                    e_pool.tile([P, NT, S], BF16, tag="e", name="y_part"))
            brc_sb = rs_chain(q)
            for hlf in range(2):
                ex_half(q, hlf, brc_sb, nxt,
                        filler=y_partial if q == NP - 1 else None)
            del e_store[q]

        # ---- finish Y with pair 7's contribution ----
        y_part = y_part_ref[0]
        for i in range(NT):
            y_ps = ps_xt.tile([P, D], F32, tag="xt", name=f"y_{i}")
            for hlf in range(2):
                nc.tensor.matmul(
                    y_ps[:, hlf * HALF:(hlf + 1) * HALF],
                    lhsT=outt_sb[:, NP - 1, i * P:(i + 1) * P],
                    rhs=wp_sb[:, NP - 1, hlf * HALF:(hlf + 1) * HALF],
                    start=True, stop=True,
                )
            y_sb = y_pool.tile([P, D], BF16, tag="y", name=f"ys_{i}")
            nc.vector.tensor_add(y_sb, y_ps, y_part[:, i, :])
            nc.sync.dma_start(d_y[i * P:(i + 1) * P, :], y_sb)


_CACHE: dict = {}


def _get_program() -> bass.Bass:
    if "nc" not in _CACHE:
        _CACHE["nc"] = build_program()
    return _CACHE["nc"]


def _prep_inputs(sin, Wx, bx, Wp, bp):
    """Host-side sharding + layout prep. Returns per-core input maps."""
    bf16 = ml_dtypes.bfloat16
    wx_flat = np.ascontiguousarray(
        np.transpose(np.asarray(Wx, np.float32), (1, 0, 2)).reshape(D, H * F)
    ).astype(bf16)
    wp_b = np.ascontiguousarray(np.asarray(Wp, np.float32)).astype(bf16)
    bx32 = np.asarray(bx, np.float32)
    # bxp[p, q] = bx[2q + p//64, p%64]
    bxp = np.ascontiguousarray(bx32.reshape(NP, P).T)
    bxf = np.ascontiguousarray(bx32.reshape(1, H * F))
    bp32 = np.ascontiguousarray(np.asarray(bp, np.float32).reshape(1, D))
    ident = np.eye(P, dtype=np.float32).astype(bf16)

    sin32 = np.asarray(sin, np.float32)
    in_maps = []
    for b in range(B):
        sint = np.ascontiguousarray(sin32[b].T).astype(bf16)
        in_maps.append({
            "sint": sint, "wx": wx_flat, "wp": wp_b, "bxp": bxp, "bxf": bxf,
            "bp": bp32, "ident": ident,
        })
    return in_maps


def kernel(sin, mask, Wx, bx, Wp, bp, _run_kwargs=None):
    nc = _get_program()
    in_maps = _prep_inputs(sin, Wx, bx, Wp, bp)
    res = run_bass_kernel_spmd(nc, in_maps, core_ids=list(range(B)),
                               **(_run_kwargs or {}))
    out = np.stack([np.asarray(res.results[b]["y"], np.float32) for b in range(B)])
    if _run_kwargs:
        _CACHE["last_results"] = res
    return out


# revision 40
# speedup vs baseline: 1.0896x; 1.0896x over previous
"""Trainium2 Bass kernel for nn_MultiHeadAttention_91027536871977.

Cosine-similarity multi-head self-attention:
  x      = einsum("bsd,hdf->bhsf", sin, Wx) + bx          [B,H,S,F]
  scores = (x @ x^T) / (|x| |x|^T)                        [B,H,S,S]
  p      = softmax(scores, -1)
  out    = concat_heads(p @ x) @ Wp + bp                  [B,S,D]

Sharding: pure data-parallel over batch (B=8 -> 8 cores, one batch each,
all 16 heads + the output projection local to the core; no collectives).

v3 (vs the 299us v1): v1's trace showed PE 80% busy dominated by ~280
small N=128 layout matmuls (transposes + selector broadcasts) and their
dependency stalls.  v3:
  - X [t,hf] via matmul as before (full-rate K=128/N=512 streams)
  - X^T per pair via a SECOND projection (lhsT = Wx blocks) instead of
    64 PE transposes; bias + 1/|x| column scaling fused into one DVE
    scalar_tensor_tensor on the PSUM result
  - the 1/|x| and 1/rs broadcast tiles ([f2,t] replication of per-token
    scalars) built by DRAM-roundtrip DMA broadcasts instead of 128
    selector matmuls on PE
  - gram/exp/out^T/Y structure kept from v1 (row-tiled K=64 gram pairs,
    col-tiled concurrent out^T pairs, exp on ACT with accum_out row sums)
  - bf16 output DMA (halves the output-drain tail; harness tolerance is
    2e-2, measured rel err ~4.4e-3)

Explored and measured SLOWER (see session notes): DVE exp polynomial in
any form (cast+accum ops get no DVE fast modes -> ~4.5-6us/tile vs ACT
1.46), quadrant 2x2 gram packing (group cadence unchanged), PSUM pool
splitting (serializes projections), xt prefetch reordering, ACT-side
squares.  HW power throttling (util capped to 0.5 for ~30-60% of
runtime, varying run to run) dominates fine-grained scheduling effects.
"""

import numpy as np
import ml_dtypes

import concourse.bass as bass
import concourse.bacc as bacc
import concourse.mybir as mybir
import concourse.tile as tile
from concourse.bass_utils import run_bass_kernel_spmd

B, S, D, H, F = 8, 1024, 1024, 16, 64
P = 128
NP = H // 2  # head pairs
KO = D // P  # k subtiles
NT = S // P  # s tiles
BF16 = mybir.dt.bfloat16
F32 = mybir.dt.float32
HALF = S // 2


def build_program() -> bass.Bass:
    nc = bacc.Bacc("TRN2", target_bir_lowering=False, debug=False)

    d_sint = nc.dram_tensor("sint", [D, S], BF16, kind="ExternalInput")
    d_wx = nc.dram_tensor("wx", [D, H * F], BF16, kind="ExternalInput")
    d_wp = nc.dram_tensor("wp", [H * F, D], BF16, kind="ExternalInput")
    d_bxp = nc.dram_tensor("bxp", [P, NP], F32, kind="ExternalInput")
    d_bxf = nc.dram_tensor("bxf", [1, H * F], F32, kind="ExternalInput")
    d_bp = nc.dram_tensor("bp", [1, D], F32, kind="ExternalInput")
    d_ident = nc.dram_tensor("ident", [P, P], BF16, kind="ExternalInput")
    d_bsc = nc.dram_tensor("bsc", [2, NP, 2 * NT, P], BF16, kind="Internal")
    d_y = nc.dram_tensor("y", [S, D], BF16, kind="ExternalOutput")

    with tile.TileContext(nc) as tc:
        _body(tc, d_sint, d_wx, d_wp, d_bxp, d_bxf, d_bp, d_ident,
              d_bsc, d_y)
    nc.compile()
    return nc


def _bcast_rows(dram_ap, parts=P, width=None):
    """DMA access pattern replicating a contiguous DRAM region across
    `parts` partitions (each partition reads the same `width` elements)."""
    if width is None:
        width = dram_ap.ap[-1][1]
    return bass.AP(
        tensor=dram_ap.tensor,
        offset=dram_ap.offset,
        ap=[[0, parts], [1, width]],
    )


def _body(tc, d_sint, d_wx, d_wp, d_bxp, d_bxf, d_bp, d_ident, d_bsc, d_y):
    nc = tc.nc
    from contextlib import ExitStack

    with ExitStack() as ctx:
        singles = ctx.enter_context(tc.tile_pool(name="singles", bufs=1))
        sq_pool = ctx.enter_context(tc.tile_pool(name="sq", bufs=2))
        e_pool = ctx.enter_context(tc.tile_pool(name="epool", bufs=4))
        b_pool = ctx.enter_context(tc.tile_pool(name="bpool", bufs=2))
        y_pool = ctx.enter_context(tc.tile_pool(name="ypool", bufs=2))
        bc_pool = ctx.enter_context(tc.tile_pool(name="bcpool", bufs=1))

        ps_big = ctx.enter_context(tc.tile_pool(name="ps_big", bufs=3, space="PSUM"))
        ps_small = ctx.enter_context(tc.tile_pool(name="ps_small", bufs=2, space="PSUM"))

        # ---- load everything to SBUF ----
        sint_sb = singles.tile([P, KO, S], BF16)
        wx_sb = singles.tile([P, KO, H * F], BF16)
        sint_r = d_sint.rearrange("(ko p) s -> p ko s", p=P)
        wx_r = d_wx.rearrange("(ko p) n -> p ko n", p=P)
        for ko in range(KO):
            nc.sync.dma_start(wx_sb[:, ko, :], wx_r[:, ko, :])
            nc.sync.dma_start(sint_sb[:, ko, :], sint_r[:, ko, :])
        wp_sb = singles.tile([P, KO, D], BF16)
        nc.sync.dma_start(wp_sb, d_wp.rearrange("(ko p) n -> p ko n", p=P))
        bxf_sb = bc_pool.tile([P, H * F], F32, tag="bc", name="bxf_sb")
        nc.gpsimd.dma_start(bxf_sb, _bcast_rows(d_bxf[:, :]))
        bxp_sb = singles.tile([P, NP], F32)
        nc.sync.dma_start(bxp_sb, d_bxp[:, :])
        ident_sb = singles.tile([P, P], BF16)
        nc.sync.dma_start(ident_sb, d_ident[:, :])

        # persistent intermediates
        x_sb = singles.tile([P, NT, H * F], BF16)   # x [t_p, t_tile, hf]
        xtn_sb = singles.tile([P, NP, S], BF16)     # normalized x^T [f2, pair, t]
        nrm_sb = singles.tile([P, NP, S], BF16)     # 1/|x| bcast [f2, pair, t]
        outt_sb = singles.tile([P, NP, S], BF16)    # attention out^T
        rs_sb = singles.tile([P, P], F32)           # rs[s_p, col h*8+i]
        n2s_sb = singles.tile([P, P], F32)          # |x|^2 [s_p, col h*8+i]
        nrcp_sb = singles.tile([P, P], F32)         # 1/|x|^2 scratch
        invs_sb = singles.tile([P, P], BF16)        # 1/|x| [s_p, col h*8+i]

        # ---- Phase A: X = sin @ Wx + bx in [t, hf] layout, per-head |x|^2 ----
        for i in range(NT):
            x_ps = ps_big.tile([P, H * F], F32, tag="big", name=f"x_{i}")
            for hlf in range(2):
                for ko in range(KO):
                    nc.tensor.matmul(
                        x_ps[:, hlf * HALF:(hlf + 1) * HALF],
                        lhsT=sint_sb[:, ko, i * P:(i + 1) * P],
                        rhs=wx_sb[:, ko, hlf * HALF:(hlf + 1) * HALF],
                        start=(ko == 0), stop=(ko == KO - 1),
                    )
            nc.vector.tensor_add(x_sb[:, i, :], x_ps, bxf_sb[:, :])
            xsq = sq_pool.tile([P, H * F], BF16, tag="xsq", name=f"xsq_{i}")
            nc.vector.tensor_mul(xsq, x_sb[:, i, :], x_sb[:, i, :])
            nc.vector.reduce_sum(
                n2s_sb.rearrange("p (hh ii) -> p hh ii", ii=NT)[:, :, i],
                xsq.rearrange("p (hh f) -> p hh f", f=F),
                axis=mybir.AxisListType.X,
            )
        nc.vector.reciprocal(nrcp_sb, n2s_sb)
        nc.scalar.sqrt(invs_sb, nrcp_sb)

        # ---- Phase A2: per-pair 1/|x| broadcast tiles via DRAM roundtrip ----
        for q in range(NP):
            invq_ps = ps_small.tile([2 * NT, P], BF16, tag="small",
                                    name=f"invq_{q}")
            nc.tensor.transpose(
                invq_ps, invs_sb[:, q * 2 * NT:(q + 1) * 2 * NT], ident_sb)
            invq_sb = b_pool.tile([2 * NT, P], BF16, tag="rcpq",
                                  name=f"invqs_{q}")
            nc.vector.tensor_copy(invq_sb, invq_ps)
            nc.sync.dma_start(d_bsc[0, q, :, :], invq_sb)
            for hh in range(2):
                nc.gpsimd.dma_start(
                    nrm_sb[hh * F:(hh + 1) * F, q, :],
                    _bcast_rows(d_bsc[0, q, hh * NT:(hh + 1) * NT, :],
                                parts=F, width=S))

        e_store = {}

        def xt_build(q):
            """X^T for pair q via direct projection; bias + normalize fused."""
            xt_ps = ps_big.tile([P, S], F32, tag="big", name=f"xt_{q}")
            for hlf in range(2):
                for ko in range(KO):
                    nc.tensor.matmul(
                        xt_ps[:, hlf * HALF:(hlf + 1) * HALF],
                        lhsT=wx_sb[:, ko, q * P:(q + 1) * P],
                        rhs=sint_sb[:, ko, hlf * HALF:(hlf + 1) * HALF],
                        start=(ko == 0), stop=(ko == KO - 1),
                    )
            nc.vector.scalar_tensor_tensor(
                xtn_sb[:, q, :], xt_ps, bxp_sb[:, q:q + 1], nrm_sb[:, q, :],
                op0=mybir.AluOpType.add, op1=mybir.AluOpType.mult,
            )

        def gram_tile(q, i):
            """Gram + exp for both heads of pair q at s-tile i."""
            g_tiles = [
                ps_big.tile([P, S], F32, tag="big", name=f"g_{q}_{hh}_{i}")
                for hh in range(2)]
            for hlf in range(2):
                for hh in range(2):
                    frows = slice(hh * F, (hh + 1) * F)
                    nc.tensor.matmul(
                        g_tiles[hh][:, hlf * HALF:(hlf + 1) * HALF],
                        lhsT=xtn_sb[frows, q, i * P:(i + 1) * P],
                        rhs=xtn_sb[frows, q, hlf * HALF:(hlf + 1) * HALF],
                        start=True, stop=True,
                    )
            for hh in range(2):
                h = 2 * q + hh
                nc.scalar.activation(
                    e_store[q][hh][:, i, :], g_tiles[hh],
                    mybir.ActivationFunctionType.Exp,
                )
                # row sums on DVE (bf16 2x mode) instead of ACT's
                # READ_ACCUMULATOR: frees ~38us of ACT-queue time
                nc.vector.reduce_sum(
                    rs_sb[:, h * NT + i:h * NT + i + 1],
                    e_store[q][hh][:, i, :],
                    axis=mybir.AxisListType.X,
                )

        def gram(q):
            e_store[q] = [
                e_pool.tile([P, NT, S], BF16, tag="e", name=f"e_{q}_{hh}")
                for hh in range(2)]
            for i in range(NT):
                gram_tile(q, i)

        def rs_chain(q):
            """1/rs broadcast tile for pair q via DRAM roundtrip."""
            rcps_sb = b_pool.tile([P, 2 * NT], F32, tag="rcps",
                                  name=f"rcps_{q}")
            nc.vector.reciprocal(
                rcps_sb, rs_sb[:, q * 2 * NT:(q + 1) * 2 * NT])
            rcpsb_sb = b_pool.tile([P, 2 * NT], BF16, tag="rcpsb",
                                   name=f"rcpsb_{q}")
            nc.vector.tensor_copy(rcpsb_sb, rcps_sb)
            rst_ps = ps_small.tile([2 * NT, P], BF16, tag="small",
                                   name=f"rst_{q}")
            nc.tensor.transpose(rst_ps, rcpsb_sb, ident_sb)
            rcpq_sb = b_pool.tile([2 * NT, P], BF16, tag="rcpq",
                                  name=f"rcpq_{q}")
            nc.vector.tensor_copy(rcpq_sb, rst_ps)
            nc.sync.dma_start(d_bsc[1, q, :, :], rcpq_sb)
            brc_sb = b_pool.tile([P, S], BF16, tag="brc", name=f"brc_{q}")
            for hh in range(2):
                nc.gpsimd.dma_start(
                    brc_sb[hh * F:(hh + 1) * F, :],
                    _bcast_rows(d_bsc[1, q, hh * NT:(hh + 1) * NT, :],
                                parts=F, width=S))
            return brc_sb

        def ex_half(q, hlf, brc_sb, nxt):
            """Half of out^T accumulation for pair q, with pair nxt's gram
            tiles interleaved to keep ACT fed."""
            ot_ps = ps_small.tile([P, HALF], F32, tag="small",
                                  name=f"ot_{q}_{hlf}")
            for j in range(NT):
                if nxt is not None and j % 2 == 0:
                    gram_tile(nxt, hlf * 4 + j // 2)
                for hh2 in range(2):
                    nc.tensor.matmul(
                        ot_ps[hh2 * F:(hh2 + 1) * F, :],
                        lhsT=x_sb[:, j, (2 * q + hh2) * F:(2 * q + hh2 + 1) * F],
                        rhs=e_store[q][hh2][:, j, hlf * HALF:(hlf + 1) * HALF],
                        start=(j == 0), stop=(j == NT - 1),
                        tile_position=(0, hh2 * F),
                        skip_group_check=True,
                    )
            nc.vector.tensor_mul(
                outt_sb[:, q, hlf * HALF:(hlf + 1) * HALF],
                brc_sb[:, hlf * HALF:(hlf + 1) * HALF],
                ot_ps,
            )

        # ---- software-pipelined attention over pairs ----
        xt_build(0)
        gram(0)
        for q in range(NP):
            nxt = q + 1 if q + 1 < NP else None
            if nxt is not None:
                xt_build(nxt)
                e_store[nxt] = [
                    e_pool.tile([P, NT, S], BF16, tag="e", name=f"e_{nxt}_{hh}")
                    for hh in range(2)]
            brc_sb = rs_chain(q)
            for hlf in range(2):
                ex_half(q, hlf, brc_sb, nxt)
            del e_store[q]

        # ---- output projection Y = out^T.T @ Wp + bp ----
        bp_sb = bc_pool.tile([P, D], F32, tag="bc", name="bp_sb")
        nc.gpsimd.dma_start(bp_sb, _bcast_rows(d_bp[:, :]))
        for i in range(NT):
            y_ps = ps_big.tile([P, D], F32, tag="big", name=f"y_{i}")
            for hlf in range(2):
                for q in range(NP):
                    nc.tensor.matmul(
                        y_ps[:, hlf * HALF:(hlf + 1) * HALF],
                        lhsT=outt_sb[:, q, i * P:(i + 1) * P],
                        rhs=wp_sb[:, q, hlf * HALF:(hlf + 1) * HALF],
                        start=(q == 0), stop=(q == NP - 1),
                    )
            y_sb = y_pool.tile([P, D], BF16, tag="y", name=f"ys_{i}")
            nc.vector.tensor_add(y_sb, y_ps, bp_sb)
            nc.sync.dma_start(d_y[i * P:(i + 1) * P, :], y_sb)


_CACHE: dict = {}


def _get_program() -> bass.Bass:
    if "nc" not in _CACHE:
        _CACHE["nc"] = build_program()
    return _CACHE["nc"]


def _prep_inputs(sin, Wx, bx, Wp, bp):
    """Host-side sharding + layout prep. Returns per-core input maps."""
    bf16 = ml_dtypes.bfloat16
    wx_flat = np.ascontiguousarray(
        np.transpose(np.asarray(Wx, np.float32), (1, 0, 2)).reshape(D, H * F)
    ).astype(bf16)
    wp_b = np.ascontiguousarray(np.asarray(Wp, np.float32)).astype(bf16)
    bx32 = np.asarray(bx, np.float32)
    # bxp[p, q] = bx[2q + p//64, p%64]
    bxp = np.ascontiguousarray(bx32.reshape(NP, P).T)
    bxf = np.ascontiguousarray(bx32.reshape(1, H * F))
    bp32 = np.ascontiguousarray(np.asarray(bp, np.float32).reshape(1, D))
    ident = np.eye(P, dtype=np.float32).astype(bf16)

    sin32 = np.asarray(sin, np.float32)
    in_maps = []
    for b in range(B):
        sint = np.ascontiguousarray(sin32[b].T).astype(bf16)
        in_maps.append({
            "sint": sint, "wx": wx_flat, "wp": wp_b, "bxp": bxp, "bxf": bxf,
            "bp": bp32, "ident": ident,
        })
    return in_maps


def kernel(sin, mask, Wx, bx, Wp, bp, _run_kwargs=None):
    nc = _get_program()
    in_maps = _prep_inputs(sin, Wx, bx, Wp, bp)
    res = run_bass_kernel_spmd(nc, in_maps, core_ids=list(range(B)),
                               **(_run_kwargs or {}))
    out = np.stack([np.asarray(res.results[b]["y"], np.float32) for b in range(B)])
    if _run_kwargs:
        _CACHE["last_results"] = res
    return out
